# revision 5
# baseline (speedup 1.0000x reference)
"""AdaptiveGlobalWeightedRankPooling2d on 8 Trainium2 NeuronCores.

Math: y[b,c] = sum_n sort_desc(x[b,c])[n] * w[c,n] / sum_n w[c,n]
with w[c,n] = sigmoid(dc_logit[c] ** n).  In f32, w[c,n] == 0.5 exactly
for n >= 18 (dc_logit ~ 0.4055); ranks >= 8 deviate by < 2e-4 in total
weight, so

    y[b,c] = sum_{j<8} top_j * wu[c,j]  +  S[b,c] * (0.5 / sum_w[c])

with wu[c,j] = (w[c,j]-0.5)/sum_w[c] host-precomputed and S the full row
sum.  x is staged to the device as bf16 (rel-err floor ~1.7e-3, gate is
2e-2).  Host-validated pipeline rel err: 2.03e-3.

Per core: 1024 rows of N=16384 bf16 = 8 tiles x [128, 16384], DMAed as
16 2MB segments into a 7-slot SBUF ring.

Engine split (measured op costs, ns):
  DVE  (0.52/elem 2x tensor_tensor; max8 = sorted top-8 at 1.04/elem):
    per tile: fold chain 16384->1024 via per-segment fold1 (2x 2288) +
    f2 2288 + f3 1221 + f4 688, then max8(f4[1024]) 1219 -> top-8 bf16.
    B-tiles additionally: 2x tensor_add pair-sum (2288 ea) -> P1.
    End: one cast top8s->f32 rall, one 96-wide mul by host weights, one
    grouped reduce -> out [128, 8].
  ScalarE (0.87/elem activation accumulate):
    A-tiles (0,1,2,4,6): ACT(raw segment, accum) x2   (14.8us/tile)
    B-tiles (3,5,7):     ACT(P1 half, accum) x2       ( 7.7us/tile)
  Projected busy: DVE ~94us, ScalarE ~97us (baseline was 123/120 ->
  125.9us total).

DVE write acks are pipelined: every DVE op is scheduled >= 2 ops after
its producer (a/b tile pair interleave) and carries a two-back vchain
wait, pre-satisfied at issue.  Cross-engine (DVE P1 -> ScalarE ACT) uses
vchain thresholds at producer+2.
"""

import numpy as np

B, C, H, W = 32, 256, 128, 128
N = H * W                 # 16384
NCORES = 8
BS = B // NCORES          # 4 batches per core
ROWS = BS * C             # 1024 rows per core
P = 128                   # partitions
NTILES = ROWS // P        # 8
SEG = 8192                # bf16 elems per segment (2MB per [128, SEG] tile)
NSEG = N // SEG           # 2 segments per tile row
NSEGS = NTILES * NSEG     # 16 segments
NSLOT = 7                 # SBUF ring depth
K = 8                     # top-K kept
RW = 12                   # rall cols per tile: 8 top + 3 sum slots + 1 pad
B_TILES = (3, 5, 7)       # tiles whose sums ride the DVE pair-add path

_CACHE = {}


def _schedule():
    """Static DVE op order + vchain position of each op (1-based, after inc)."""
    ops = []
    for k in range(NTILES // 2):
        a, b = 2 * k, 2 * k + 1
        if a == 0:
            ops += [("f1q0", 0), ("f1q1", 0)]
        else:
            ops += [("f1L", a)]
        ops += [("f1R", a), ("f1L", b), ("f1R", b)]
        for t in (a, b):
            if t in B_TILES:
                ops += [("addL", t), ("addR", t)]
        for kind in ("f2", "f3", "f4", "m8"):
            ops += [(kind, a), (kind, b)]
    ops += [("cast", -1), ("mul", -1), ("red", -1)]
    pos = {op: i + 1 for i, op in enumerate(ops)}
    return ops, pos


def _build():
    if "nc_raw" in _CACHE:
        return _CACHE["nc_raw"]
    from concourse import bacc, mybir

    f32 = mybir.dt.float32
    bf16 = mybir.dt.bfloat16
    Copy = mybir.ActivationFunctionType.Copy
    X = mybir.AxisListType.X
    add = mybir.AluOpType.add

    ops, pos = _schedule()
    V_TOTAL = len(ops)

    # ScalarE emission (tile order); achain count after each tile's ACTs.
    # tile0: 3 ACTs (two seg0 halves + seg1); A-tiles: 2; B-tiles: 2.
    acnt = 0
    a_after_tile = {}
    for t in range(NTILES):
        acnt += 3 if t == 0 else 2
        a_after_tile[t] = acnt
    A_TOTAL = acnt

    # per-seg frees (vchain / achain) for ring reuse
    v_free = {}
    a_free = {}
    arun = 0
    for t in range(NTILES):
        sL, sR = 2 * t, 2 * t + 1
        if t in B_TILES:
            v_free[sL] = pos[("addL", t)]
            v_free[sR] = pos[("addR", t)]
        elif t == 0:
            v_free[sL] = pos[("f1q1", 0)]
            v_free[sR] = pos[("f1R", 0)]
        else:
            v_free[sL] = pos[("f1L", t)]
            v_free[sR] = pos[("f1R", t)]
        if t == 0:
            a_free[sL] = 2   # after the two half-ACTs
            a_free[sR] = 3
            arun = 3
        elif t not in B_TILES:
            a_free[sL] = arun + 1
            a_free[sR] = arun + 2
            arun += 2
        else:
            # B-tile slots are only read by DVE (covered by v_free)
            a_free[sL] = arun
            a_free[sR] = arun
            arun += 2

    nc = bacc.Bacc(
        "TRN2", target_bir_lowering=False, debug=False, num_devices=NCORES
    )
    x = nc.dram_tensor("x", [ROWS, N], bf16, kind="ExternalInput").ap()
    cpk = nc.dram_tensor("cpk", [P, NTILES * RW], f32, kind="ExternalInput").ap()
    out = nc.dram_tensor("out", [P, NTILES], f32, kind="ExternalOutput").ap()
    import os
    dbg = bool(os.environ.get("KERNEL_DEBUG_RALL"))
    rall_out = (
        nc.dram_tensor("rall_out", [P, NTILES * RW], f32, kind="ExternalOutput").ap()
        if dbg
        else None
    )

    xbuf = nc.alloc_sbuf_tensor("xbuf", [P, NSLOT * SEG], bf16).ap()
    m1a = nc.alloc_sbuf_tensor("m1a", [P, SEG], bf16).ap()
    m1b = nc.alloc_sbuf_tensor("m1b", [P, SEG], bf16).ap()
    f2a = nc.alloc_sbuf_tensor("f2a", [P, SEG // 2], bf16).ap()
    f2b = nc.alloc_sbuf_tensor("f2b", [P, SEG // 2], bf16).ap()
    f3a = nc.alloc_sbuf_tensor("f3a", [P, SEG // 4], bf16).ap()
    f3b = nc.alloc_sbuf_tensor("f3b", [P, SEG // 4], bf16).ap()
    f4a = nc.alloc_sbuf_tensor("f4a", [P, SEG // 8], bf16).ap()
    f4b = nc.alloc_sbuf_tensor("f4b", [P, SEG // 8], bf16).ap()
    p1 = nc.alloc_sbuf_tensor("p1", [P, SEG], bf16).ap()
    dummy = nc.alloc_sbuf_tensor("actdummy", [P, SEG], bf16).ap()
    cand = nc.alloc_sbuf_tensor("cand", [P, NTILES * K], bf16).ap()
    rall = nc.alloc_sbuf_tensor("rall", [P, NTILES * RW], f32).ap()
    scr = nc.alloc_sbuf_tensor("scr", [P, NTILES * RW], f32).ap()
    cpksb = nc.alloc_sbuf_tensor("cpksb", [P, NTILES * RW], f32).ap()
    outsb = nc.alloc_sbuf_tensor("outsb", [P, NTILES], f32).ap()

    seg_sem = [nc.alloc_semaphore(f"seg{k}") for k in range(NSLOT)]
    seg0a_sem = nc.alloc_semaphore("seg0a")
    cst_sem = nc.alloc_semaphore("cst")
    mset_sem = nc.alloc_semaphore("mset")
    out_sem = nc.alloc_semaphore("outd")
    vchain = nc.alloc_semaphore("vchain")
    achain = nc.alloc_semaphore("achain")

    def seg_thresh(i):
        return 16 * (i // NSLOT + 1)

    def slot(i):
        return xbuf[:, (i % NSLOT) * SEG : (i % NSLOT + 1) * SEG]

    # achain value the DVE pair-add of B-tile t must wait for (P1 free)
    prev_b = {5: 3, 7: 5}

    with nc.Block(no_gpsimd_drain=True) as block:

        @block.sync
        def _(sync):
            for i in range(NSEGS):
                if i >= NSLOT:
                    j = i - NSLOT
                    sync.wait_ge(vchain, v_free[j])
                    sync.wait_ge(achain, a_free[j])
                if i == 0:
                    sync.dma_start(
                        out=xbuf[:, 0 : SEG // 2], in_=x[0:P, 0 : SEG // 2]
                    ).then_inc(seg0a_sem, 16)
                    sync.dma_start(
                        out=xbuf[:, SEG // 2 : SEG], in_=x[0:P, SEG // 2 : SEG]
                    ).then_inc(seg_sem[0], 16)
                else:
                    t, sg = divmod(i, NSEG)
                    sync.dma_start(
                        out=slot(i),
                        in_=x[t * P : (t + 1) * P, sg * SEG : (sg + 1) * SEG],
                    ).then_inc(seg_sem[i % NSLOT], 16)
            sync.wait_ge(vchain, V_TOTAL)
            sync.dma_start(out=out[:], in_=outsb[:]).then_inc(out_sem, 16)
            if dbg:
                sync.dma_start(out=rall_out[:], in_=rall[:]).then_inc(out_sem, 16)
            sync.wait_ge(out_sem, 32 if dbg else 16)

        @block.gpsimd
        def _(g):
            g.memset(rall[:], 0.0).then_inc(mset_sem, 1)

        @block.scalar
        def _(s):
            s.dma_start(out=cpksb[:], in_=cpk[:]).then_inc(cst_sem, 16)
            s.wait_ge(mset_sem, 1)
            n = 0

            def act(src, col, sem=None, val=None, vwait=None):
                nonlocal n
                if sem is not None:
                    s.wait_ge(sem, val)
                if vwait is not None:
                    s.wait_ge(vchain, vwait)
                ins = s.activation(
                    dummy[:, 0 : src.shape[1]],
                    src,
                    Copy,
                    bias=0.0,
                    scale=1.0,
                    accum_out=rall[:, col : col + 1],
                )
                if n >= 2:
                    ins._wait_ge(achain, n - 1)
                ins.then_inc(achain)
                n += 1

            for t in range(NTILES):
                rb = t * RW
                kL, kR = (2 * t) % NSLOT, (2 * t + 1) % NSLOT
                if t == 0:
                    act(xbuf[:, 0 : SEG // 2], rb + K, seg0a_sem, 16)
                    act(xbuf[:, SEG // 2 : SEG], rb + K + 1, seg_sem[0], 16)
                    act(slot(1), rb + K + 2, seg_sem[1], 16)
                elif t not in B_TILES:
                    act(slot(2 * t), rb + K, seg_sem[kL], seg_thresh(2 * t))
                    act(slot(2 * t + 1), rb + K + 1, seg_sem[kR],
                        seg_thresh(2 * t + 1))
                else:
                    act(p1[:, 0 : SEG // 2], rb + K,
                        vwait=pos[("addL", t)] + 2)
                    act(p1[:, SEG // 2 : SEG], rb + K + 1,
                        vwait=pos[("addR", t)] + 2)
            assert n == A_TOTAL, (n, A_TOTAL)

        @block.vector
        def _(v):
            vc = 0

            def emit(ins):
                nonlocal vc
                ins._wait_ge(vchain, max(0, vc - 1))
                ins.then_inc(vchain)
                vc += 1

            def emit_serial(ins):
                # producer is the immediately preceding op: wait its write ack
                nonlocal vc
                ins._wait_ge(vchain, vc)
                ins.then_inc(vchain)
                vc += 1

            m1 = {0: m1a, 1: m1b}
            f2 = {0: f2a, 1: f2b}
            f3 = {0: f3a, 1: f3b}
            f4 = {0: f4a, 1: f4b}

            v.wait_ge(cst_sem, 16)
            v.wait_ge(mset_sem, 1)

            for kind, t in ops:
                h = t % 2  # a/b buffer set
                if kind == "f1q0":
                    v.wait_ge(seg0a_sem, 16)
                    emit(v.tensor_max(
                        m1a[:, 0 : SEG // 4],
                        xbuf[:, 0 : SEG // 4],
                        xbuf[:, SEG // 4 : SEG // 2],
                    ))
                elif kind == "f1q1":
                    v.wait_ge(seg_sem[0], 16)
                    emit(v.tensor_max(
                        m1a[:, SEG // 4 : SEG // 2],
                        xbuf[:, SEG // 2 : SEG // 2 + SEG // 4],
                        xbuf[:, SEG // 2 + SEG // 4 : SEG],
                    ))
                elif kind in ("f1L", "f1R", "addL", "addR"):
                    sg = 2 * t + (0 if kind.endswith("L") else 1)
                    kslot = sg % NSLOT
                    if kind.startswith("f1"):
                        v.wait_ge(seg_sem[kslot], seg_thresh(sg))
                    if kind == "addL" and t in prev_b:
                        v.wait_ge(achain, a_after_tile[prev_b[t]])
                    src = slot(sg)
                    off = 0 if kind.endswith("L") else SEG // 2
                    if kind.startswith("f1"):
                        ins = v.tensor_max(
                            m1[h][:, off : off + SEG // 2],
                            src[:, 0 : SEG // 2],
                            src[:, SEG // 2 : SEG],
                        )
                    else:
                        ins = v.tensor_add(
                            p1[:, off : off + SEG // 2],
                            src[:, 0 : SEG // 2],
                            src[:, SEG // 2 : SEG],
                        )
                    emit(ins)
                elif kind == "f2":
                    emit(v.tensor_max(
                        f2[h][:], m1[h][:, 0 : SEG // 2], m1[h][:, SEG // 2 : SEG]
                    ))
                elif kind == "f3":
                    emit(v.tensor_max(
                        f3[h][:], f2[h][:, 0 : SEG // 4], f2[h][:, SEG // 4 : SEG // 2]
                    ))
                elif kind == "f4":
                    emit(v.tensor_max(
                        f4[h][:], f3[h][:, 0 : SEG // 8], f3[h][:, SEG // 8 : SEG // 4]
                    ))
                elif kind == "m8":
                    emit(v.max(cand[:, t * K : (t + 1) * K], f4[h][:]))
                elif kind == "cast":
                    # top8 bf16 -> f32 into rall cols t*RW..t*RW+7 (strided)
                    emit_serial(v.tensor_copy(
                        rall.rearrange("p (t r) -> p t r", r=RW)[:, :, 0:K],
                        cand.rearrange("p (t k) -> p t k", k=K),
                    ))
                elif kind == "mul":
                    v.wait_ge(achain, A_TOTAL)
                    emit_serial(v.tensor_mul(scr[:], rall[:], cpksb[:]))
                elif kind == "red":
                    emit_serial(v.tensor_reduce(
                        outsb[:],
                        scr.rearrange("p (t r) -> p t r", r=RW),
                        axis=X,
                        op=add,
                    ))
            assert vc == V_TOTAL, (vc, V_TOTAL)

    nc.compile()
    _CACHE["nc_raw"] = nc
    return nc


def _host_weights(dc_logit: np.ndarray):
    """wu2[c, 0:8] = (w[c,j]-0.5)/sum_w[c]; cols 8..10 = 0.5/sum_w; col 11 = 0."""
    dc = dc_logit.astype(np.float64)
    j = np.arange(N, dtype=np.float64)
    pw = dc[:, None] ** j[None, :]
    wfull = (1.0 / (1.0 + np.exp(-pw))).astype(np.float32)  # [C, N]
    dev = np.abs(wfull[:, K:] - np.float32(0.5))
    assert dev.max() < 2e-4, f"top-{K} decomposition invalid: {dev.max()}"
    sum_w = wfull.astype(np.float64).sum(axis=1)
    winv = (1.0 / sum_w).astype(np.float64)
    wu2 = np.zeros((C, RW), np.float32)
    wu2[:, :K] = ((wfull[:, :K].astype(np.float64) - 0.5) * winv[:, None]).astype(
        np.float32
    )
    half = (0.5 * winv).astype(np.float32)
    for c in range(K, K + 3):
        wu2[:, c] = half
    return wu2


def _run_pjrt(nc, in_maps):
    """Pre-uploads all inputs to the devices before dispatching the NEFF."""
    import jax
    import numpy as np
    from jax.sharding import Mesh, NamedSharding, PartitionSpec
    from jax.experimental.shard_map import shard_map
    from concourse import bass2jax, mybir

    bass2jax.install_neuronx_cc_hook()
    assert nc.dbg_addr is None
    n_cores = len(in_maps)
    partition_name = (
        nc.partition_id_tensor.name if nc.partition_id_tensor else None
    )

    in_names, out_names, out_avals, zero_outs = [], [], [], []
    for alloc in nc.m.functions[0].allocations:
        if not isinstance(alloc, mybir.MemoryLocationSet):
            continue
        name = alloc.memorylocations[0].name
        if alloc.kind == "ExternalInput":
            if name != partition_name:
                in_names.append(name)
        elif alloc.kind == "ExternalOutput":
            shape = tuple(alloc.tensor_shape)
            dtype = mybir.dt.np(alloc.dtype)
            out_names.append(name)
            out_avals.append(jax.core.ShapedArray(shape, dtype))
            zero_outs.append(np.zeros(shape, dtype))
    n_params = len(in_names)
    n_outs = len(out_avals)
    all_in_names = list(in_names) + out_names
    if partition_name is not None:
        all_in_names.append(partition_name)
    donate = tuple(range(n_params, n_params + n_outs))

    def _body(*args):
        operands = list(args)
        if partition_name is not None:
            operands.append(bass2jax.partition_id_tensor())
        return tuple(
            bass2jax._bass_exec_p.bind(
                *operands,
                out_avals=tuple(out_avals),
                in_names=tuple(all_in_names),
                out_names=tuple(out_names),
                lowering_input_output_aliases=(),
                sim_require_finite=True,
                sim_require_nnan=True,
                nc=nc,
            )
        )

    devices = jax.devices()[:n_cores]
    mesh = Mesh(np.asarray(devices), ("core",))
    spec = PartitionSpec("core")
    sharded = jax.jit(
        shard_map(
            _body,
            mesh=mesh,
            in_specs=(spec,) * (n_params + n_outs),
            out_specs=(spec,) * n_outs,
            check_rep=False,
        ),
        donate_argnums=donate,
        keep_unused=True,
    )
    sh = NamedSharding(mesh, spec)
    concat_in = [
        jax.device_put(
            np.concatenate([np.asarray(in_maps[c][k]) for c in range(n_cores)], axis=0),
            sh,
        )
        for k in in_names
    ]
    concat_zeros = [
        jax.device_put(
            np.zeros((n_cores * z.shape[0], *z.shape[1:]), z.dtype), sh
        )
        for z in zero_outs
    ]
    jax.block_until_ready(concat_in)
    jax.block_until_ready(concat_zeros)
    out_arrs = sharded(*concat_in, *concat_zeros)
    return [
        {
            name: np.asarray(out_arrs[i]).reshape(n_cores, *out_avals[i].shape)[c]
            for i, name in enumerate(out_names)
        }
        for c in range(n_cores)
    ]


def _in_maps(x: np.ndarray, dc_logit: np.ndarray):
    import ml_dtypes

    wu2 = _host_weights(np.asarray(dc_logit))  # [C, RW]
    # cpk pre-tiled: col t*RW + j -> wu2[(t%2)*128 + p, j]
    cpk = np.empty((P, NTILES * RW), np.float32)
    for t in range(NTILES):
        cpk[:, t * RW : (t + 1) * RW] = wu2[(t % 2) * P : (t % 2 + 1) * P]
    xr = np.ascontiguousarray(x).reshape(B * C, N).astype(ml_dtypes.bfloat16)
    return [
        {"x": xr[i * ROWS : (i + 1) * ROWS], "cpk": cpk}
        for i in range(NCORES)
    ]


def kernel(x: np.ndarray, dc_logit: np.ndarray) -> np.ndarray:
    import time

    nc = _build()
    in_maps = _in_maps(x, dc_logit)
    last_err = None
    for attempt in range(3):
        try:
            results = _run_pjrt(nc, in_maps)
            break
        except Exception as e:  # transient device errors (wedged core etc.)
            last_err = e
            time.sleep(15)
    else:
        raise last_err
    outs = []
    for i in range(NCORES):
        o = results[i]["out"]  # [P, NTILES]; col t, row p -> global row t*128+p
        outs.append(o.T.reshape(BS, C))
    return np.concatenate(outs, axis=0).astype(np.float32)
